# revision 8
# baseline (speedup 1.0000x reference)
import os
from contextlib import ExitStack

import numpy as np

import concourse.bass as bass
import concourse.mybir as mybir
import concourse.tile as tile
from concourse import bacc
from concourse.bass_utils import run_bass_kernel_spmd
from concourse.masks import make_identity

f32 = mybir.dt.float32
i16 = mybir.dt.int16

N_NODES = 100000
DIM = 64
L = 5
NCORES = 8
NS = N_NODES // NCORES
BLK = 4096
T = BLK // 128
SINGLE_PACKET = False

LAST_RESULTS = None

_PROGRAM_CACHE = {}


def _build_program(S: int):
    ncalls_per_seg = S // BLK
    nc = bacc.Bacc("TRN2", target_bir_lowering=False, debug=False, num_devices=NCORES)

    zu = nc.dram_tensor("zu", [NS, DIM], f32, kind="ExternalInput").ap()
    zm = nc.dram_tensor("zm", [N_NODES, DIM], f32, kind="ExternalInput").ap()
    wf = nc.dram_tensor("wf", [DIM, L * DIM], f32, kind="ExternalInput").ap()
    nidx = NCORES * S
    si = nc.dram_tensor("si", [128, nidx // 16], i16, kind="ExternalInput").ap()
    di = nc.dram_tensor("di", [128, nidx // 16], i16, kind="ExternalInput").ap()
    out = nc.dram_tensor("scores", [nidx, L], f32, kind="ExternalOutput").ap()

    with tile.TileContext(nc) as tc:
        with ExitStack() as ctx:
            singles = ctx.enter_context(tc.tile_pool(name="singles", bufs=1))
            gpool = ctx.enter_context(tc.tile_pool(name="gpool", bufs=2))
            work = ctx.enter_context(tc.tile_pool(name="work", bufs=3))
            spool = ctx.enter_context(tc.tile_pool(name="spool", bufs=2))
            ps_t = ctx.enter_context(tc.tile_pool(name="ps_t", bufs=2, space="PSUM"))
            ps_y = ctx.enter_context(tc.tile_pool(name="ps_y", bufs=2, space="PSUM"))

            ident = singles.tile([128, 128], f32)
            make_identity(nc, ident)
            wf_sb = singles.tile([DIM, L * DIM], f32)
            nc.sync.dma_start(out=wf_sb, in_=wf)

            si_sb = singles.tile([128, nidx // 16], i16)
            nc.sync.dma_start(out=si_sb, in_=si)
            di_sb = singles.tile([128, nidx // 16], i16)
            nc.sync.dma_start(out=di_sb, in_=di)

            for seg in range(NCORES):
                zm_seg = zm[seg * NS : (seg + 1) * NS, :]
                for call_i in range(ncalls_per_seg):
                    call = seg * ncalls_per_seg + call_i
                    c0, c1 = call * (BLK // 16), (call + 1) * (BLK // 16)
                    zs_g = gpool.tile([128, T, DIM], f32, tag="zs")
                    nc.gpsimd.dma_gather(zs_g, zu, si_sb[:, c0:c1], BLK, BLK, DIM, single_packet=SINGLE_PACKET)
                    zd_g = gpool.tile([128, T, DIM], f32, tag="zd")
                    nc.gpsimd.dma_gather(zd_g, zm_seg, di_sb[:, c0:c1], BLK, BLK, DIM, single_packet=SINGLE_PACKET)
                    s_sb = spool.tile([128, T, L], f32)
                    for t in range(T):
                        zsT_ps = ps_t.tile([DIM, 128], f32, tag="zsT")
                        nc.tensor.transpose(zsT_ps, zs_g[:, t, :], ident)
                        zsT_sb = work.tile([DIM, 128], f32, tag="zsT_sb")
                        nc.vector.tensor_copy(zsT_sb, zsT_ps)

                        y_ps = ps_y.tile([128, L * DIM], f32, tag="y")
                        nc.tensor.matmul(y_ps, zsT_sb, wf_sb)

                        p_sb = work.tile([128, L, DIM], f32, tag="p")
                        zd_t = zd_g[:, t, :]
                        zd_b = bass.AP(
                            tensor=zd_t.tensor,
                            offset=zd_t.offset,
                            ap=[list(zd_t.ap[0]), [0, L], list(zd_t.ap[1])],
                        )
                        nc.vector.tensor_mul(
                            p_sb, y_ps.rearrange("p (l f) -> p l f", l=L), zd_b
                        )
                        nc.vector.tensor_reduce(
                            s_sb[:, t, :],
                            p_sb,
                            axis=mybir.AxisListType.X,
                            op=mybir.AluOpType.add,
                        )
                    nc.sync.dma_start(
                        out=out[call * BLK : (call + 1) * BLK, :].rearrange(
                            "(t p) l -> p t l", p=128
                        ),
                        in_=s_sb,
                    )

    nc.compile()
    return nc


def _plan(src, dst):
    E = src.shape[0]
    core = np.minimum(src // NS, NCORES - 1)
    blk = np.minimum(dst // NS, NCORES - 1)
    order = np.lexsort((blk, core))
    counts = np.bincount(core * NCORES + blk, minlength=NCORES * NCORES).reshape(
        NCORES, NCORES
    )
    max_cell = int(counts.max())
    S = ((max_cell + BLK - 1) // BLK) * BLK

    src_s, dst_s, ids_s = src[order], dst[order], order
    si = np.zeros((NCORES, NCORES * S), np.int16)
    di = np.zeros((NCORES, NCORES * S), np.int16)
    ids = np.full((NCORES, NCORES * S), -1, np.int64)
    cell_starts = np.zeros((NCORES, NCORES), np.int64)
    flat = 0
    for c in range(NCORES):
        for j in range(NCORES):
            n = counts[c, j]
            sel = slice(flat, flat + n)
            pos = j * S
            si[c, pos : pos + n] = (src_s[sel] - c * NS).astype(np.int16)
            di[c, pos : pos + n] = (dst_s[sel] - j * NS).astype(np.int16)
            ids[c, pos : pos + n] = ids_s[sel]
            cell_starts[c, j] = flat
            flat += n
    assert flat == E

    def wrap(a):
        return np.ascontiguousarray(np.tile(a.reshape(-1, 16).T, (8, 1)))

    si_w = np.stack([wrap(si[c]) for c in range(NCORES)])
    di_w = np.stack([wrap(di[c]) for c in range(NCORES)])
    return S, si_w, di_w, ids


def kernel(**inputs) -> np.ndarray:
    global LAST_RESULTS
    z_user = np.ascontiguousarray(np.asarray(inputs["z_user"], dtype=np.float32))
    z_movie = np.ascontiguousarray(np.asarray(inputs["z_movie"], dtype=np.float32))
    W = np.asarray(inputs["W"], dtype=np.float32)
    eli = np.asarray(inputs["edge_label_index"])
    src = eli[0].astype(np.int64)
    dst = eli[1].astype(np.int64)
    E = src.shape[0]

    S, si_w, di_w, ids = _plan(src, dst)
    wf = np.ascontiguousarray(W.transpose(1, 0, 2).reshape(DIM, L * DIM))

    if S not in _PROGRAM_CACHE:
        _PROGRAM_CACHE[S] = _build_program(S)
    nc = _PROGRAM_CACHE[S]

    in_maps = []
    for c in range(NCORES):
        in_maps.append(
            {
                "zu": np.ascontiguousarray(z_user[c * NS : (c + 1) * NS]),
                "zm": z_movie,
                "wf": wf,
                "si": si_w[c],
                "di": di_w[c],
            }
        )

    trace = bool(int(os.environ.get("KERNEL_TRACE", "0")))
    res = run_bass_kernel_spmd(
        nc, in_maps, core_ids=list(range(NCORES)), trace=trace
    )
    LAST_RESULTS = res

    scores = np.empty((E, L), np.float32)
    for c in range(NCORES):
        out_c = res.results[c]["scores"]
        m = ids[c] >= 0
        scores[ids[c][m]] = out_c[m]

    mx = scores.max(axis=0, keepdims=True)
    lse = np.log(np.exp(scores - mx).sum(axis=0, keepdims=True)) + mx
    return scores - lse
